# revision 22
# baseline (speedup 1.0000x reference)
"""Multi-head attention (B=2, P=2048, DIM=1024, H=16, d=64) on 8 trn2 cores.

Sharding: core c = 4*b + g handles batch b = c//4 and heads 4g..4g+3 (g = c%4),
for the full 2048-token sequence.

Design (local-proj + 4-core ReduceScatter; no AllToAll):
  - Inputs x^T and W_qkv arrive bf16 (host-converted); QKV projection for the
    core's 4 heads in transposed layout (Q^T/K^T: [dh, seq]) off x^T.
    Schedule: V first (all 16 seq-chunks, gated by the x^T DMA), plus Q/K for
    q-block 0; Q/K for later q-blocks ride as fillers inside earlier rounds'
    PE idle, so the ACT-bound round pipeline starts as early as possible.
  - Attention per head in S^T orientation, q-block-major rounds (qc outer,
    head inner): S^T tiles [128k, 512q], exp on ScalarE (scale 1/8 folded),
    AV matmul with V augmented by a ones column (M=65) so the softmax
    denominator lands in PSUM row 64. Normalize with DVE reciprocal +
    gpsimd partition_broadcast; normalized O^T written straight into SBUF
    (og_s, head-pair-stacked layout) - no DRAM staging.
  - After each q-block's 4 heads: local partial output projection
    O^T[qblock] @ W_proj[rows of own heads] + bias/4 -> partial [512, 1024]
    bf16; a 4-core ReduceScatter(add) over the batch group sums the 4 cores'
    partials and hands each core a distinct 128-row q-slice, DMA'd straight
    to the (bf16) output. The 4 RS calls pipeline with later rounds; proj
    matmuls ride as fillers between S groups (psum slots borrowed from the
    st ring, which is always-safe to wait on).
  - Core c emits rows qc*128..+127 = global q rows qc*512 + (c%4)*128..+127.
"""

import sys

sys.path.insert(0, "/opt/trn_rl_repo")

import numpy as np
import concourse.bass as bass
import concourse.tile as tile
import concourse.mybir as mybir
from concourse import bacc
from concourse.bass import ts
from concourse.bass_utils import run_bass_kernel_spmd

FP = mybir.dt.float32
BF = mybir.dt.bfloat16
N_CORES = 8
B, P, DIM, H, D = 2, 2048, 1024, 16, 64
HPC = H // 4  # heads per core = 4
DHC = HPC * D  # dh per core = 256
NQ = P // 512  # 4 q-blocks of 512
NK = P // 128  # 16 k-chunks of 128
ND = DIM // 128  # 8 dim-chunks
G = 3  # k-chunks per exp group (psum tile banks)
V_PRE = 8  # V chunks computed pre-round (rest ride as fillers)
MM_DT = mybir.dt.float32r  # matmul operand dtype (fp32r: 4x faster PE)
EX_DT = mybir.dt.bfloat16  # exp output / AV moving operand dtype
RG4 = [[0, 1, 2, 3], [4, 5, 6, 7]]

_CACHE = {}


def _build(repeat=1, stop_after=None, fake_cc=False):
    nc = bacc.Bacc(
        "TRN2",
        target_bir_lowering=False,
        debug=False,
        enable_asserts=False,
        num_devices=N_CORES,
    )
    xt = nc.dram_tensor("xt", [DIM, P], BF, kind="ExternalInput").ap()
    wq = nc.dram_tensor("wq", [DIM, DHC], BF, kind="ExternalInput").ap()
    wk = nc.dram_tensor("wk", [DIM, DHC], BF, kind="ExternalInput").ap()
    wv = nc.dram_tensor("wv", [DIM, DHC], BF, kind="ExternalInput").ap()
    wp = nc.dram_tensor("wp", [DHC, DIM], FP, kind="ExternalInput").ap()
    # bias/4: each core folds a quarter of the bias into its partial so the
    # 4-way ReduceScatter(add) reconstitutes the full bias.
    bias = nc.dram_tensor("bias", [128, DIM], FP, kind="ExternalInput").ap()
    out = nc.dram_tensor("out", [NQ * 128, DIM], BF, kind="ExternalOutput").ap()

    with tile.TileContext(nc) as tc:
        with (
            tc.tile_pool(name="s1", bufs=1) as s1,
            tc.tile_pool(name="es", bufs=7) as es,
            tc.tile_pool(name="wk2", bufs=2) as wk2,
            tc.tile_pool(name="ob", bufs=2) as ob,
            tc.tile_pool(name="dram", bufs=1, space="DRAM") as dram,
            tc.tile_pool(name="spool", bufs=2, space="PSUM") as spool,
            tc.tile_pool(name="avpool", bufs=2, space="PSUM") as avpool,
        ):
            qt_s = s1.tile([128, 2, P], MM_DT)
            kt_s = s1.tile([128, 2, NK, 128], MM_DT)
            v_s = s1.tile([128, NK, HPC, D + 1], EX_DT)
            bias_s = s1.tile([128, DIM], FP)
            nc.sync.dma_start(bias_s[:], bias[:])
            nc.vector.memset(v_s[:, :, :, D : D + 1], 1.0)

            prt = [dram.tile([NQ * 128, DIM], BF, name=f"prt{i}") for i in range(NQ)]
            rso = [dram.tile([128, DIM], BF, name=f"rso{i}") for i in range(NQ)]

            def one_pass():
              with tc.tile_pool(name="s2", bufs=1) as s2:
                xt_s = s2.tile([128, ND, P], BF)
                wq_s = s2.tile([128, ND, DHC], BF)
                wk_s = s2.tile([128, ND, DHC], BF)
                wv_s = s2.tile([128, ND, DHC], BF)
                wp_s = s2.tile([128, 2, DIM], MM_DT)
                og_s = s2.tile([128, 2, NQ, 512], MM_DT)

                # -- loads, in first-use order ---------------------------
                wql = wq.rearrange("(c p) n -> p c n", p=128)
                for dc in range(ND):
                    nc.sync.dma_start(wq_s[:, dc, :], wql[:, dc, :])
                    nc.sync.dma_start(
                        xt_s[:, dc, 0:1024], xt[ts(dc, 128), 0:1024]
                    )
                nc.sync.dma_start(
                    wk_s[:], wk.rearrange("(c p) n -> p c n", p=128)
                )
                nc.sync.dma_start(
                    wv_s[:], wv.rearrange("(c p) n -> p c n", p=128)
                )
                for dc in range(ND):
                    nc.sync.dma_start(
                        xt_s[:, dc, 1024:2048], xt[ts(dc, 128), 1024:2048]
                    )
                nc.sync.dma_start(
                    wp_s[:], wp.rearrange("(c p) n -> p c n", p=128).bitcast(MM_DT)
                )

                # -- QKV pieces ------------------------------------------
                def qk_half(j, qc, which, acc, part):
                    """acc: [128, 512] psum region. part 0/1: dc 0-3 / 4-7."""
                    w_s = wq_s if which == "q" else wk_s
                    for dc in range(4 * part, 4 * part + 4):
                        nc.tensor.matmul(
                            acc,
                            w_s[:, dc, ts(j, 128)],
                            xt_s[:, dc, ts(qc, 512)],
                            start=(dc == 0),
                            stop=(dc == ND - 1),
                        )
                    if part == 1:
                        if which == "q":
                            nc.vector.tensor_copy(
                                out=qt_s[:, j, ts(qc, 512)], in_=acc
                            )
                        else:
                            nc.vector.tensor_copy(
                                out=kt_s[:, j, 4 * qc : 4 * qc + 4, :],
                                in_=acc.rearrange("p (a b) -> p a b", b=128),
                            )

                def v_chunk(sc, acc):
                    """acc: [128, 512] psum region (first DHC cols used)."""
                    for dc in range(ND):
                        nc.tensor.matmul(
                            acc[:, 0:DHC],
                            xt_s[:, dc, ts(sc, 128)],
                            wv_s[:, dc, :],
                            start=(dc == 0),
                            stop=(dc == ND - 1),
                        )
                    nc.vector.tensor_copy(
                        out=v_s[:, sc, :, 0:D],
                        in_=acc[:, 0:DHC].rearrange("p (h d) -> p h d", d=D),
                    )

                # Pre-round QKV work (avpool ping-pong is safe here: no av
                # accumulators yet). K^T spans the full sequence and is read
                # by every round, so K(j=0) is computed upfront (K(j=1) rides
                # as fillers in rounds 0-1, which only use j=0); Q only for
                # q-block 0; V chunks 0-7 (8-15 ride as fillers).
                def pre_piece(fn):
                    ps = avpool.tile([128, 512], FP, tag="ps", name="pre")
                    fn(ps[:])

                def qk_full(j, qc, which, acc):
                    qk_half(j, qc, which, acc, 0)
                    qk_half(j, qc, which, acc, 1)

                pre_piece(lambda a: qk_full(0, 0, "q", a))
                pre_piece(lambda a: qk_full(1, 0, "q", a))
                for s in range(4):
                    pre_piece(lambda a, s=s: qk_full(0, s, "k", a))
                for sc in range(V_PRE):
                    pre_piece(lambda a, sc=sc: v_chunk(sc, a))

                if stop_after == "qkv":
                    nc.sync.dma_start(out[0:128, 0:512], qt_s[:, 0, 0:256].bitcast(BF))
                    return

                # -- rounds ----------------------------------------------
                groups = [(k0, min(k0 + G, NK)) for k0 in range(0, NK, G)]

                import collections as _c

                pend = _c.deque()  # (h, av, ex, k0, k1, tail_info|None)

                def emit_tail(h, qc, av):
                    rec = wk2.tile([1, 512], FP, tag="rec", name="rec")
                    nc.vector.reciprocal(rec[:], av[D : D + 1, :])
                    bc = wk2.tile([64, 512], FP, tag="bc", name="bc")
                    nc.gpsimd.partition_broadcast(bc[:], rec[:])
                    hp = 64 * (h % 2)
                    nc.vector.tensor_mul(
                        og_s[hp : hp + 64, h // 2, qc, :], av[0:D, :], bc[:]
                    )

                def flush_one():
                    h_, av_, ex_, k0_, k1_, tinfo = pend.popleft()
                    for k in range(k0_, k1_):
                        nc.tensor.matmul(
                            av_[0 : D + 1, :],
                            v_s[:, k, h_, :],
                            ex_[:, k - k0_, :],
                            start=(k == 0),
                            stop=(k == NK - 1),
                            skip_group_check=True,
                        )
                    if tinfo is not None:
                        emit_tail(tinfo[0], tinfo[1], av_)

                av_cur = [None]

                def round_groups(h, qc, filler):
                    j, hp = h // 2, 64 * (h % 2)
                    for gi, (k0, k1) in enumerate(groups):
                        st = spool.tile([128, G, 512], FP, tag="st", name="st")
                        for k in range(k0, k1):
                            nc.tensor.matmul(
                                st[:, k - k0, :],
                                kt_s[hp : hp + 64, j, k, :],
                                qt_s[hp : hp + 64, j, ts(qc, 512)],
                                start=True,
                                stop=True,
                            )
                        ex = es.tile([128, G, 512], EX_DT, tag="ex", name="ex")
                        nc.scalar.activation(
                            out=ex[:, 0 : k1 - k0, :],
                            in_=st[:, 0 : k1 - k0, :],
                            func=mybir.ActivationFunctionType.Exp,
                            scale=float(D) ** -0.5,
                        )
                        if gi == 0:
                            av_cur[0] = avpool.tile(
                                [128, 512], FP, tag="ps", name="av"
                            )
                        pend.append(
                            (
                                h,
                                av_cur[0],
                                ex,
                                k0,
                                k1,
                                (h, qc) if gi == len(groups) - 1 else None,
                            )
                        )
                        if filler:
                            filler.popleft()()
                        while len(pend) > 2:
                            flush_one()

                # -- proj + RS pieces ------------------------------------
                obuf_cur = {}

                def emit_pso(qc, sc, od):
                    # psum slot borrowed from the st ring (bank 0 of the
                    # 3-bank slot); slot-waits are against long-retired exp
                    # reads, never a live av accumulator. bias/4 pre-folded.
                    if qc not in obuf_cur:
                        obuf_cur[qc] = ob.tile(
                            [128, 4, DIM], BF, tag="obuf", name="obuf"
                        )
                    pso = spool.tile([128, G, 512], FP, tag="st", name="pso")
                    for j in range(2):
                        nc.tensor.matmul(
                            pso[:, 0, :],
                            og_s[:, j, qc, ts(sc, 128)],
                            wp_s[:, j, ts(od, 512)],
                            start=(j == 0),
                            stop=(j == 1),
                        )
                    nc.vector.tensor_add(
                        obuf_cur[qc][:, sc, ts(od, 512)],
                        pso[:, 0, :],
                        bias_s[:, ts(od, 512)],
                    )

                def emit_rs(qc):
                    obuf = obuf_cur.pop(qc)
                    nc.sync.dma_start(
                        prt[qc].rearrange("(s p) n -> p s n", p=128), obuf[:]
                    )
                    if fake_cc:
                        nc.sync.dma_start(rso[qc][:], prt[qc][0:128, :])
                    else:
                        nc.gpsimd.collective_compute(
                            "ReduceScatter",
                            mybir.AluOpType.add,
                            replica_groups=RG4,
                            ins=[prt[qc].opt()],
                            outs=[rso[qc].opt()],
                        )
                    # final store: plain DRAM->DRAM copy (bf16 out)
                    nc.sync.dma_start(out[ts(qc, 128), :], rso[qc][:])

                # -- filler queue & main loop ----------------------------
                fill = _c.deque()

                def fill_qk(j, qc, which):
                    def go():
                        acc = spool.tile([128, G, 512], FP, tag="st", name="pqk")
                        qk_full(j, qc, which, acc[:, 0, :])

                    fill.append(go)

                def queue_proj(qc):
                    for i in range(8):
                        sc, od = divmod(i, 2)
                        fill.append(lambda qc=qc, sc=sc, od=od: emit_pso(qc, sc, od))
                    fill.append(lambda qc=qc: emit_rs(qc))

                def v_fill(sc):
                    psv = spool.tile([128, G, 512], FP, tag="st", name="psv")
                    v_chunk(sc, psv[:, 0, :])

                # block-0 fillers: V 8-15 (AV consumes k-chunks in order, and
                # fillers run before the pend flush, so v(k) always lands
                # ahead of the AV group that reads it), then K(j=1) needed
                # from round (2,0) on.
                for sc in range(V_PRE, NK):
                    fill.append(lambda sc=sc: v_fill(sc))
                for s in range(4):
                    fill_qk(1, s, "k")

                for qc in range(NQ):
                    for h in range(4):
                        round_groups(h, qc, fill)
                        if h == 0 and qc >= 1:
                            queue_proj(qc - 1)
                        if h == 1 and qc < NQ - 1:
                            fill_qk(0, qc + 1, "q")
                            fill_qk(1, qc + 1, "q")
                while pend:
                    flush_one()
                while fill:
                    fill.popleft()()
                if stop_after == "rounds":
                    nc.sync.dma_start(
                        out[0:128, 0:512], og_s[:, 0, 0, 0:256].bitcast(BF)
                    )
                    return
                queue_proj(NQ - 1)
                while fill:
                    fill.popleft()()

            for _rep in range(repeat):
                one_pass()

    nc.compile()
    return nc


def _prep_inputs(x, W_qkv, W_proj, b_proj):
    """Host-side sharding: per-core input dicts."""
    import ml_dtypes

    bf = ml_dtypes.bfloat16
    x = np.asarray(x, dtype=np.float32)
    W_qkv = np.asarray(W_qkv, dtype=np.float32)
    W_proj = np.asarray(W_proj, dtype=np.float32)
    b_proj = np.asarray(b_proj, dtype=np.float32)

    # bias/4: summed back to full bias by the 4-way ReduceScatter(add)
    bias_b = np.ascontiguousarray(
        np.broadcast_to(b_proj[None, :] * 0.25, (128, DIM)).astype(np.float32)
    )
    xb = [np.ascontiguousarray(x[b].T).astype(bf) for b in range(B)]
    W8 = W_qkv.astype(bf)
    in_maps = []
    for c in range(N_CORES):
        b, g = divmod(c, 4)
        wq = np.ascontiguousarray(W8[:, 0 * DIM + DHC * g : 0 * DIM + DHC * (g + 1)])
        wk = np.ascontiguousarray(W8[:, 1 * DIM + DHC * g : 1 * DIM + DHC * (g + 1)])
        wv = np.ascontiguousarray(W8[:, 2 * DIM + DHC * g : 2 * DIM + DHC * (g + 1)])
        wp = np.ascontiguousarray(W_proj[DHC * g : DHC * (g + 1), :])  # [256, DIM]
        in_maps.append(
            {"xt": xb[b], "wq": wq, "wk": wk, "wv": wv, "wp": wp, "bias": bias_b}
        )
    return in_maps


def kernel(x, W_qkv, W_proj, b_proj, _trace=False, _tmpdir=None):
    if "nc" not in _CACHE:
        _CACHE["nc"] = _build()
    nc = _CACHE["nc"]
    in_maps = _prep_inputs(x, W_qkv, W_proj, b_proj)
    res = run_bass_kernel_spmd(
        nc,
        in_maps,
        core_ids=list(range(N_CORES)),
        trace=_trace,
        tmpdir=_tmpdir,
        stitch_traces=False,
    )
    _CACHE["last_results"] = res
    full = np.empty((B, P, DIM), dtype=np.float32)
    for c in range(N_CORES):
        b, m = divmod(c, 4)
        # [512, DIM] bf16, block qc at rows qc*128
        o = np.asarray(res.results[c]["out"]).astype(np.float32)
        for qc in range(NQ):
            r0 = qc * 512 + m * 128
            full[b, r0 : r0 + 128, :] = o[qc * 128 : (qc + 1) * 128]
    return full


# revision 23
# speedup vs baseline: 1.4116x; 1.4116x over previous
"""Multi-head attention (B=2, P=2048, DIM=1024, H=16, d=64) on 8 trn2 cores.

Sharding: core c = 4*b + g handles batch b = c//4 and heads 4g..4g+3 (g = c%4),
for the full 2048-token sequence.

Design (local-proj + 4-core ReduceScatter; no AllToAll):
  - Inputs x^T and W_qkv arrive bf16 (host-converted); QKV projection for the
    core's 4 heads in transposed layout (Q^T/K^T: [dh, seq]) off x^T.
    Schedule: V first (all 16 seq-chunks, gated by the x^T DMA), plus Q/K for
    q-block 0; Q/K for later q-blocks ride as fillers inside earlier rounds'
    PE idle, so the ACT-bound round pipeline starts as early as possible.
  - Attention per head in S^T orientation, q-block-major rounds (qc outer,
    head inner): S^T tiles [128k, 512q], exp on ScalarE (scale 1/8 folded),
    AV matmul with V augmented by a ones column (M=65) so the softmax
    denominator lands in PSUM row 64. Normalize with DVE reciprocal +
    gpsimd partition_broadcast; normalized O^T written straight into SBUF
    (og_s, head-pair-stacked layout) - no DRAM staging.
  - After each q-block's 4 heads: local partial output projection
    O^T[qblock] @ W_proj[rows of own heads] + bias/4 -> partial [512, 1024]
    bf16; a 4-core ReduceScatter(add) over the batch group sums the 4 cores'
    partials and hands each core a distinct 128-row q-slice, DMA'd straight
    to the (bf16) output. The 4 RS calls pipeline with later rounds; proj
    matmuls ride as fillers between S groups (psum slots borrowed from the
    st ring, which is always-safe to wait on).
  - Core c emits rows qc*128..+127 = global q rows qc*512 + (c%4)*128..+127.
"""

import sys

sys.path.insert(0, "/opt/trn_rl_repo")

import numpy as np
import concourse.bass as bass
import concourse.tile as tile
import concourse.mybir as mybir
from concourse import bacc
from concourse.bass import ts
from concourse.bass_utils import run_bass_kernel_spmd

FP = mybir.dt.float32
BF = mybir.dt.bfloat16
N_CORES = 8
B, P, DIM, H, D = 2, 2048, 1024, 16, 64
HPC = H // 4  # heads per core = 4
DHC = HPC * D  # dh per core = 256
NQ = P // 512  # 4 q-blocks of 512
NK = P // 128  # 16 k-chunks of 128
ND = DIM // 128  # 8 dim-chunks
G = 3  # k-chunks per exp group (psum tile banks)
V_PRE = 8  # V chunks computed pre-round (rest ride as fillers)
MM_DT = mybir.dt.float32r  # matmul operand dtype (fp32r: 4x faster PE)
EX_DT = mybir.dt.bfloat16  # exp output / AV moving operand dtype
RG4 = [[0, 1, 2, 3], [4, 5, 6, 7]]

_CACHE = {}


def _build(repeat=1, stop_after=None, fake_cc=False):
    nc = bacc.Bacc(
        "TRN2",
        target_bir_lowering=False,
        debug=False,
        enable_asserts=False,
        num_devices=N_CORES,
    )
    xt = nc.dram_tensor("xt", [DIM, P], BF, kind="ExternalInput").ap()
    wq = nc.dram_tensor("wq", [DIM, DHC], BF, kind="ExternalInput").ap()
    wk = nc.dram_tensor("wk", [DIM, DHC], BF, kind="ExternalInput").ap()
    wv = nc.dram_tensor("wv", [DIM, DHC], BF, kind="ExternalInput").ap()
    wp = nc.dram_tensor("wp", [DHC, DIM], FP, kind="ExternalInput").ap()
    # bias/4: each core folds a quarter of the bias into its partial so the
    # 4-way ReduceScatter(add) reconstitutes the full bias.
    bias = nc.dram_tensor("bias", [128, DIM], FP, kind="ExternalInput").ap()
    out = nc.dram_tensor("out", [NQ * 128, DIM], BF, kind="ExternalOutput").ap()

    with tile.TileContext(nc) as tc:
        with (
            tc.tile_pool(name="s1", bufs=1) as s1,
            tc.tile_pool(name="es", bufs=7) as es,
            tc.tile_pool(name="wk2", bufs=2) as wk2,
            tc.tile_pool(name="ob", bufs=2) as ob,
            tc.tile_pool(name="dram", bufs=1, space="DRAM") as dram,
            tc.tile_pool(name="spool", bufs=2, space="PSUM") as spool,
            tc.tile_pool(name="avpool", bufs=2, space="PSUM") as avpool,
        ):
            qt_s = s1.tile([128, 2, P], MM_DT)
            kt_s = s1.tile([128, 2, NK, 128], MM_DT)
            v_s = s1.tile([128, NK, HPC, D + 1], EX_DT)
            bias_s = s1.tile([128, DIM], FP)
            nc.sync.dma_start(bias_s[:], bias[:])
            nc.vector.memset(v_s[:, :, :, D : D + 1], 1.0)

            prt = [dram.tile([NQ * 128, DIM], BF, name=f"prt{i}") for i in range(NQ)]
            rso = [dram.tile([128, DIM], BF, name=f"rso{i}") for i in range(NQ)]

            def one_pass():
              with tc.tile_pool(name="s2", bufs=1) as s2:
                xt_s = s2.tile([128, ND, P], BF)
                wq_s = s2.tile([128, ND, DHC], BF)
                wk_s = s2.tile([128, ND, DHC], BF)
                wv_s = s2.tile([128, ND, DHC], BF)
                wp_s = s2.tile([128, 2, DIM], MM_DT)
                og_s = s2.tile([128, 2, NQ, 512], MM_DT)

                # -- loads, in first-use order ---------------------------
                wql = wq.rearrange("(c p) n -> p c n", p=128)
                for dc in range(ND):
                    nc.sync.dma_start(wq_s[:, dc, :], wql[:, dc, :])
                    nc.sync.dma_start(
                        xt_s[:, dc, 0:1024], xt[ts(dc, 128), 0:1024]
                    )
                nc.sync.dma_start(
                    wk_s[:], wk.rearrange("(c p) n -> p c n", p=128)
                )
                nc.sync.dma_start(
                    wv_s[:], wv.rearrange("(c p) n -> p c n", p=128)
                )
                for dc in range(ND):
                    nc.sync.dma_start(
                        xt_s[:, dc, 1024:2048], xt[ts(dc, 128), 1024:2048]
                    )
                nc.sync.dma_start(
                    wp_s[:], wp.rearrange("(c p) n -> p c n", p=128).bitcast(MM_DT)
                )

                # -- QKV pieces ------------------------------------------
                def qk_half(j, qc, which, acc, part):
                    """acc: [128, 512] psum region. part 0/1: dc 0-3 / 4-7."""
                    w_s = wq_s if which == "q" else wk_s
                    for dc in range(4 * part, 4 * part + 4):
                        nc.tensor.matmul(
                            acc,
                            w_s[:, dc, ts(j, 128)],
                            xt_s[:, dc, ts(qc, 512)],
                            start=(dc == 0),
                            stop=(dc == ND - 1),
                        )
                    if part == 1:
                        if which == "q":
                            nc.vector.tensor_copy(
                                out=qt_s[:, j, ts(qc, 512)], in_=acc
                            )
                        else:
                            nc.vector.tensor_copy(
                                out=kt_s[:, j, 4 * qc : 4 * qc + 4, :],
                                in_=acc.rearrange("p (a b) -> p a b", b=128),
                            )

                def v_chunk(sc, acc):
                    """acc: [128, 512] psum region (first DHC cols used)."""
                    for dc in range(ND):
                        nc.tensor.matmul(
                            acc[:, 0:DHC],
                            xt_s[:, dc, ts(sc, 128)],
                            wv_s[:, dc, :],
                            start=(dc == 0),
                            stop=(dc == ND - 1),
                        )
                    nc.vector.tensor_copy(
                        out=v_s[:, sc, :, 0:D],
                        in_=acc[:, 0:DHC].rearrange("p (h d) -> p h d", d=D),
                    )

                # Pre-round QKV work (avpool ping-pong is safe here: no av
                # accumulators yet). K^T spans the full sequence and is read
                # by every round, so K(j=0) is computed upfront (K(j=1) rides
                # as fillers in rounds 0-1, which only use j=0); Q only for
                # q-block 0; V chunks 0-7 (8-15 ride as fillers).
                def pre_piece(fn):
                    ps = avpool.tile([128, 512], FP, tag="ps", name="pre")
                    fn(ps[:])

                def qk_full(j, qc, which, acc):
                    qk_half(j, qc, which, acc, 0)
                    qk_half(j, qc, which, acc, 1)

                pre_piece(lambda a: qk_full(0, 0, "q", a))
                pre_piece(lambda a: qk_full(1, 0, "q", a))
                for s in range(4):
                    pre_piece(lambda a, s=s: qk_full(0, s, "k", a))
                for sc in range(V_PRE):
                    pre_piece(lambda a, sc=sc: v_chunk(sc, a))

                if stop_after == "qkv":
                    nc.sync.dma_start(out[0:128, 0:512], qt_s[:, 0, 0:256].bitcast(BF))
                    return

                # -- rounds ----------------------------------------------
                groups = [(k0, min(k0 + G, NK)) for k0 in range(0, NK, G)]

                import collections as _c

                pend = _c.deque()  # (h, av, ex, k0, k1, tail_info|None)

                def emit_tail(h, qc, av):
                    rec = wk2.tile([1, 512], FP, tag="rec", name="rec")
                    nc.vector.reciprocal(rec[:], av[D : D + 1, :])
                    bc = wk2.tile([64, 512], FP, tag="bc", name="bc")
                    nc.gpsimd.partition_broadcast(bc[:], rec[:])
                    hp = 64 * (h % 2)
                    nc.vector.tensor_mul(
                        og_s[hp : hp + 64, h // 2, qc, :], av[0:D, :], bc[:]
                    )

                def flush_one():
                    h_, av_, ex_, k0_, k1_, tinfo = pend.popleft()
                    for k in range(k0_, k1_):
                        nc.tensor.matmul(
                            av_[0 : D + 1, :],
                            v_s[:, k, h_, :],
                            ex_[:, k - k0_, :],
                            start=(k == 0),
                            stop=(k == NK - 1),
                            skip_group_check=True,
                        )
                    if tinfo is not None:
                        emit_tail(tinfo[0], tinfo[1], av_)

                av_cur = [None]

                def round_groups(h, qc, filler):
                    j, hp = h // 2, 64 * (h % 2)
                    for gi, (k0, k1) in enumerate(groups):
                        st = spool.tile([128, G, 512], FP, tag="st", name="st")
                        for k in range(k0, k1):
                            nc.tensor.matmul(
                                st[:, k - k0, :],
                                kt_s[hp : hp + 64, j, k, :],
                                qt_s[hp : hp + 64, j, ts(qc, 512)],
                                start=True,
                                stop=True,
                            )
                        ex = es.tile([128, G, 512], EX_DT, tag="ex", name="ex")
                        nc.scalar.activation(
                            out=ex[:, 0 : k1 - k0, :],
                            in_=st[:, 0 : k1 - k0, :],
                            func=mybir.ActivationFunctionType.Exp,
                            scale=float(D) ** -0.5,
                        )
                        if gi == 0:
                            av_cur[0] = avpool.tile(
                                [128, 512], FP, tag="ps", name="av"
                            )
                        pend.append(
                            (
                                h,
                                av_cur[0],
                                ex,
                                k0,
                                k1,
                                (h, qc) if gi == len(groups) - 1 else None,
                            )
                        )
                        if filler:
                            filler.popleft()()
                        while len(pend) > 2:
                            flush_one()

                # -- proj + RS pieces ------------------------------------
                obuf_cur = {}

                def emit_pso(qc, sc, od):
                    # psum slot borrowed from the st ring (bank 0 of the
                    # 3-bank slot); slot-waits are against long-retired exp
                    # reads, never a live av accumulator. bias/4 pre-folded.
                    if qc not in obuf_cur:
                        obuf_cur[qc] = ob.tile(
                            [128, 4, DIM], BF, tag="obuf", name="obuf"
                        )
                    pso = spool.tile([128, G, 512], FP, tag="st", name="pso")
                    for j in range(2):
                        nc.tensor.matmul(
                            pso[:, 0, :],
                            og_s[:, j, qc, ts(sc, 128)],
                            wp_s[:, j, ts(od, 512)],
                            start=(j == 0),
                            stop=(j == 1),
                        )
                    nc.vector.tensor_add(
                        obuf_cur[qc][:, sc, ts(od, 512)],
                        pso[:, 0, :],
                        bias_s[:, ts(od, 512)],
                    )

                def emit_rs(qc):
                    obuf = obuf_cur.pop(qc)
                    nc.sync.dma_start(
                        prt[qc].rearrange("(s p) n -> p s n", p=128), obuf[:]
                    )
                    # fake_cc: True = all RS faked; int k = fake the first k
                    nfake = 4 if fake_cc is True else int(fake_cc or 0)
                    if qc < nfake:
                        nc.sync.dma_start(rso[qc][:], prt[qc][0:128, :])
                    else:
                        nc.gpsimd.collective_compute(
                            "ReduceScatter",
                            mybir.AluOpType.add,
                            replica_groups=RG4,
                            ins=[prt[qc].opt()],
                            outs=[rso[qc].opt()],
                        )
                    # final store: plain DRAM->DRAM copy (bf16 out)
                    nc.sync.dma_start(out[ts(qc, 128), :], rso[qc][:])

                # -- filler queue & main loop ----------------------------
                fill = _c.deque()

                def fill_qk(j, qc, which):
                    def go():
                        acc = spool.tile([128, G, 512], FP, tag="st", name="pqk")
                        qk_full(j, qc, which, acc[:, 0, :])

                    fill.append(go)

                def queue_proj(qc):
                    for i in range(8):
                        sc, od = divmod(i, 2)
                        fill.append(lambda qc=qc, sc=sc, od=od: emit_pso(qc, sc, od))
                    fill.append(lambda qc=qc: emit_rs(qc))

                def v_fill(sc):
                    psv = spool.tile([128, G, 512], FP, tag="st", name="psv")
                    v_chunk(sc, psv[:, 0, :])

                # block-0 fillers: V 8-15 (AV consumes k-chunks in order, and
                # fillers run before the pend flush, so v(k) always lands
                # ahead of the AV group that reads it), then K(j=1) needed
                # from round (2,0) on.
                for sc in range(V_PRE, NK):
                    fill.append(lambda sc=sc: v_fill(sc))
                for s in range(4):
                    fill_qk(1, s, "k")

                for qc in range(NQ):
                    for h in range(4):
                        round_groups(h, qc, fill)
                        if h == 0 and qc >= 1:
                            queue_proj(qc - 1)
                        if h == 1 and qc < NQ - 1:
                            fill_qk(0, qc + 1, "q")
                            fill_qk(1, qc + 1, "q")
                while pend:
                    flush_one()
                while fill:
                    fill.popleft()()
                if stop_after == "rounds":
                    nc.sync.dma_start(
                        out[0:128, 0:512], og_s[:, 0, 0, 0:256].bitcast(BF)
                    )
                    return
                queue_proj(NQ - 1)
                while fill:
                    fill.popleft()()

            for _rep in range(repeat):
                one_pass()

    nc.compile()
    return nc


def _prep_inputs(x, W_qkv, W_proj, b_proj):
    """Host-side sharding: per-core input dicts."""
    import ml_dtypes

    bf = ml_dtypes.bfloat16
    x = np.asarray(x, dtype=np.float32)
    W_qkv = np.asarray(W_qkv, dtype=np.float32)
    W_proj = np.asarray(W_proj, dtype=np.float32)
    b_proj = np.asarray(b_proj, dtype=np.float32)

    # bias/4: summed back to full bias by the 4-way ReduceScatter(add)
    bias_b = np.ascontiguousarray(
        np.broadcast_to(b_proj[None, :] * 0.25, (128, DIM)).astype(np.float32)
    )
    xb = [np.ascontiguousarray(x[b].T).astype(bf) for b in range(B)]
    W8 = W_qkv.astype(bf)
    in_maps = []
    for c in range(N_CORES):
        b, g = divmod(c, 4)
        wq = np.ascontiguousarray(W8[:, 0 * DIM + DHC * g : 0 * DIM + DHC * (g + 1)])
        wk = np.ascontiguousarray(W8[:, 1 * DIM + DHC * g : 1 * DIM + DHC * (g + 1)])
        wv = np.ascontiguousarray(W8[:, 2 * DIM + DHC * g : 2 * DIM + DHC * (g + 1)])
        wp = np.ascontiguousarray(W_proj[DHC * g : DHC * (g + 1), :])  # [256, DIM]
        in_maps.append(
            {"xt": xb[b], "wq": wq, "wk": wk, "wv": wv, "wp": wp, "bias": bias_b}
        )
    return in_maps


def kernel(x, W_qkv, W_proj, b_proj, _trace=False, _tmpdir=None):
    if "nc" not in _CACHE:
        _CACHE["nc"] = _build()
    nc = _CACHE["nc"]
    in_maps = _prep_inputs(x, W_qkv, W_proj, b_proj)
    res = run_bass_kernel_spmd(
        nc,
        in_maps,
        core_ids=list(range(N_CORES)),
        trace=_trace,
        tmpdir=_tmpdir,
        stitch_traces=False,
    )
    _CACHE["last_results"] = res
    full = np.empty((B, P, DIM), dtype=np.float32)
    for c in range(N_CORES):
        b, m = divmod(c, 4)
        # [512, DIM] bf16, block qc at rows qc*128
        o = np.asarray(res.results[c]["out"]).astype(np.float32)
        for qc in range(NQ):
            r0 = qc * 512 + m * 128
            full[b, r0 : r0 + 128, :] = o[qc * 128 : (qc + 1) * 128]
    return full
